# revision 38
# baseline (speedup 1.0000x reference)
"""Trainium2 Bass kernel for a dense transformer block (LN -> QKV -> attention ->
out-proj -> LN -> FFN with exact GELU, no residuals).

Sharding: pure data parallelism — batch 8 across 8 NeuronCores, one batch element
per core. Each core runs the full block on its [1024, 1024] token slab.

On-chip dataflow (per core):
  - LN1 in token-major fp32 (bn_stats); normalized output cast to bf16 and
    transposed to feature-major xnT [D, tokens] on the PE (bf16 transpose-mode).
  - All GEMMs in bf16 with fp32 PSUM accumulation. LayerNorm gains fold into the
    weights on the host; LN biases fold into per-feature GEMM biases. The
    1/sqrt(dh) attention scale folds into the Q projection.
  - Q/K produced feature-major per head pair (streamed), V token-major with a
    ones-column per head so attention@V also yields the softmax denominator.
  - Scores are computed k-major (scoresT) so the exp output feeds attention@V
    directly with no transpose; softmax skips max-subtraction (|scores| < ~3).
  - Denominator reciprocal is partition-broadcast by GPSIMD (no DMA).
  - x2 token-major, LN2 like LN1, FFN1 feature-major with fused bias+GELU on
    ACT, FFN2 token-major.

DMA discipline (hardware caps each DMA descriptor at ~2 sem waits, and Tile
adds one flow-control wait per DMA once queues saturate, so every DMA must
carry at most ONE data wait):
  - Weights live in 4 permanent 16KB "wslot" tag slots (wq/wk/wv/wout ->
    w1 quarters -> w2 quarters). Between tenants the slot is fully memset by
    GPSIMD: engine instructions have no wait caps and become the slot's sole
    last-writer, so the next load carries a single WAW wait.
  - Activation pools recycle SBUF freely but are only ever written by compute
    engines; output staging is a permanent pool so stores see only DVE deps.
"""

import numpy as np
import ml_dtypes

B, N, D = 8, 1024, 1024
H, DH = 16, 64
MLP = 4096
EPS = 1e-5
P = 128
NCORES = 8
TT = N // P    # 8 token tiles
DC = D // P    # 8 d-chunks
MT = MLP // P  # 32 mlp tiles


def build_bass(gelu_mode="gelu"):
    import concourse.bass as bass
    import concourse.mybir as mybir
    import concourse.tile as tile
    from concourse import bacc
    from concourse.masks import make_identity

    f32 = mybir.dt.float32
    bf16 = mybir.dt.bfloat16
    AF = mybir.ActivationFunctionType
    OP = mybir.AluOpType

    nc = bacc.Bacc()

    x_d = nc.declare_dram_parameter("x", [N, D], bf16, isOutput=False)
    wq_d = nc.declare_dram_parameter("wq", [D, D], bf16, isOutput=False)
    wk_d = nc.declare_dram_parameter("wk", [D, D], bf16, isOutput=False)
    wv_d = nc.declare_dram_parameter("wv", [D, D], bf16, isOutput=False)
    wo_d = nc.declare_dram_parameter("wo", [D, D], bf16, isOutput=False)
    w1_d = nc.declare_dram_parameter("w1", [D, MLP], bf16, isOutput=False)
    w2_d = nc.declare_dram_parameter("w2", [MLP, D], bf16, isOutput=False)
    bq_d = nc.declare_dram_parameter("bq", [D], f32, isOutput=False)
    bk_d = nc.declare_dram_parameter("bk", [D], f32, isOutput=False)
    bv_d = nc.declare_dram_parameter("bv", [D], f32, isOutput=False)
    bo_d = nc.declare_dram_parameter("bo", [D], f32, isOutput=False)
    bh_d = nc.declare_dram_parameter("bh", [MLP], f32, isOutput=False)
    b2_d = nc.declare_dram_parameter("b2", [D], f32, isOutput=False)
    out_d = nc.declare_dram_parameter("out", [N, D], f32, isOutput=True)

    gelu_func = AF.Gelu if gelu_mode == "gelu" else AF.Identity

    with tile.TileContext(nc) as tc:
        # ---- permanent pools (left stack bottom) ----
        const = tc.alloc_tile_pool(name="const", bufs=1)
        stats = tc.alloc_tile_pool(name="stats", bufs=4)
        psum = tc.alloc_tile_pool(name="psum", bufs=8, space="PSUM")
        wslot = tc.alloc_tile_pool(name="wslot", bufs=4)   # 4 x 16KB weight slots
        outp = tc.alloc_tile_pool(name="outp", bufs=2)

        counter = [0]

        def uniq(prefix):
            counter[0] += 1
            return f"{prefix}{counter[0]}"

        def ps_tile():
            return psum.tile([P, 512], f32, tag="ps", bufs=4, name=uniq("ps"))

        def pse_tile():
            # 2-bank [P,1024] tile: a row-tiled matmul pair writes its halves
            # (each within one bank) and a single wide ACT drains both.
            return psum.tile([P, 1024], f32, tag="pse", bufs=2,
                             name=uniq("pse"))

        def wtile(shape):
            return wslot.tile(shape, bf16, tag="w", name=uniq("w"))

        def qpad(n, after_ap=None):
            """Emit n dummy HWDGE DMAs (virgin tiny targets) to rotate the
            round-robin queue assignment, so the next real load lands on the
            same queue as its slot's previous tenant load (merging the
            slot-WAW wait with the prior-same-queue wait). after_ap pins the
            pads' schedule position via a RAW dependency."""
            for _ in range(n):
                if after_ap is None:
                    d = const.tile([1, 4], f32, tag=uniq("pad"),
                                   name=uniq("padt"))
                    nc.sync.dma_start(d, bq_d[None, 0:4])
                else:
                    d = const.tile([1, 4], bf16, tag=uniq("pad"),
                                   name=uniq("padt"))
                    nc.sync.dma_start(d, after_ap)

        eps_t = const.tile([P, 1], f32, tag="eps")
        nc.vector.memset(eps_t, EPS)
        bv_b = const.tile([P, D], bf16, tag="bvb")
        nc.gpsimd.dma_start(bv_b, bv_d[None, :].to_broadcast([P, D]))
        ident = const.tile([P, P], bf16, tag="ident")
        make_identity(nc, ident)
        # selector for the denominator broadcast: bones.T @ den_r replicates
        # den_r row 0 onto output partitions 0:64 and row 64 onto 64:128
        bonesf = const.tile([65, P], f32, tag="bonesf")
        nc.vector.memset(bonesf, 0.0)
        nc.vector.memset(bonesf[0:1, 0:64], 1.0)
        nc.vector.memset(bonesf[64:65, 64:128], 1.0)
        bones = const.tile([65, P], mybir.dt.float32r, tag="bones")
        nc.vector.tensor_copy(bones, bonesf)

        def layer_norm_tile(x_t, xn_t):
            """token-major [128, D] fp32 -> normalized bf16 (no gain/bias)."""
            nc.vector.memset(xn_t[0:1, 0:4], 0.0)   # claim slot: absorb WAR deps
            st = stats.tile([P, 2, 6], f32, tag="st", name=uniq("st"))
            xr = x_t.rearrange("p (s d) -> p s d", s=2)
            nc.vector.bn_stats(st[:, 0], xr[:, 0])
            nc.vector.bn_stats(st[:, 1], xr[:, 1])
            mv = stats.tile([P, 2], f32, tag="mv", name=uniq("mv"))
            nc.vector.bn_aggr(mv, st)
            rstd = stats.tile([P, 1], f32, tag="rstd", name=uniq("rstd"))
            nc.scalar.activation(rstd, mv[:, 1:2], func=AF.Sqrt, bias=eps_t,
                                 scale=1.0)
            nc.vector.reciprocal(rstd, rstd)
            nc.vector.tensor_scalar(xn_t, x_t, scalar1=mv[:, 0:1], scalar2=rstd,
                                    op0=OP.subtract, op1=OP.mult)

        def pe_transpose_to(dstT, ti, src_t):
            """[128 tok, D] bf16 -> feature-major dstT[:, dj, ti-slice].
            4 transposes share one PSUM tile so a single wide ACT copy
            drains them (one 352-cycle ACT overhead per 4, not per 1)."""
            for g in range(2):
                pst = psum.tile([P, 4, P], bf16, tag="ps", bufs=4,
                                name=uniq("pst"))
                for k in range(4):
                    dj = g * 4 + k
                    nc.tensor.transpose(pst[:, k, :],
                                        src_t[:, dj * P:(dj + 1) * P], ident)
                nc.scalar.activation(
                    dstT[:, g * 4:(g + 1) * 4, ti * P:(ti + 1) * P], pst,
                    func=AF.Copy)

        # ---- phase 1: LN1 + transpose to feature-major ----
        # left-stack pools in reverse-death order (LIFO); x slab on the right
        lnxn = tc.alloc_tile_pool(name="lnxn", bufs=2)     # dies after LN2
        aop = tc.alloc_tile_pool(name="ao", bufs=1)
        denp = tc.alloc_tile_pool(name="den", bufs=1)
        x2p = tc.alloc_tile_pool(name="x2", bufs=2)
        xnTp = tc.alloc_tile_pool(name="xnT", bufs=1)
        vap = tc.alloc_tile_pool(name="vaug", bufs=1)
        qkp = tc.alloc_tile_pool(name="qk", bufs=4)
        expp = tc.alloc_tile_pool(name="expp", bufs=18)
        lnx = tc.alloc_tile_pool(name="lnx", bufs=1, side="right")

        xnT = xnTp.tile([P, DC, N], bf16, tag="xnT")
        xfull = lnx.tile([P, TT, D], bf16, tag="x")
        # x tiles FIRST (LN1 blocks on tile 0), then weights in first-use
        # order (wv for the fused V projection, wq/wk at attention, wo last)
        # the sync HWDGE queue is SERIAL (~0.6us min per DMA) -- one DMA per
        # x tile, in strict first-need order: x0..x7, wv (V projection),
        # then wq/wk (attention) and wo; tiny bias loads last
        for xq in range(TT):
            nc.sync.dma_start(
                xfull[:, xq:xq + 1, :],
                x_d[xq * P:(xq + 1) * P, :].rearrange(
                    "(o p) f -> p o f", p=P))
        wv_sb = wtile([P, DC, D])
        nc.sync.dma_start(wv_sb, wv_d[:, :].rearrange("(o p) f -> p o f", p=P))
        wq_sb = wtile([P, DC, D])
        nc.sync.dma_start(wq_sb, wq_d[:, :].rearrange("(o p) f -> p o f", p=P))
        wk_sb = wtile([P, DC, D])
        nc.sync.dma_start(wk_sb, wk_d[:, :].rearrange("(o p) f -> p o f", p=P))
        wo_sb = wtile([P, DC, D])
        nc.sync.dma_start(wo_sb, wo_d[:, :].rearrange("(o p) f -> p o f", p=P))
        bq_sb = const.tile([P, DC], f32, tag="bq")
        nc.sync.dma_start(bq_sb, bq_d[:].rearrange("(o p) -> p o", p=P))
        bk_sb = const.tile([P, DC], f32, tag="bk")
        nc.sync.dma_start(bk_sb, bk_d[:].rearrange("(o p) -> p o", p=P))
        bh_sb = const.tile([P, MT], f32, tag="bh")
        nc.sync.dma_start(bh_sb, bh_d[:].rearrange("(o p) -> p o", p=P))
        bo_b = const.tile([P, D], bf16, tag="bob")
        nc.gpsimd.dma_start(bo_b, bo_d[None, :].to_broadcast([P, D]))
        b2_b = const.tile([P, D], bf16, tag="b2b")
        nc.gpsimd.dma_start(b2_b, b2_d[None, :].to_broadcast([P, D]))
        # V projection for tile ti needs only tile ti's transposes, so it is
        # fused one iteration behind LN1: the PE runs V matmuls while the DVE
        # does the next tile's LayerNorm (PE-dense from kernel start).
        v_aug = vap.tile([P, TT, H, DH + 1], bf16, tag="vaug")
        nc.vector.memset(v_aug[:, :, :, DH:DH + 1], 1.0)

        def emit_v(ti):
            for fh in range(2):
                psv = ps_tile()
                for dc in range(DC):
                    nc.tensor.matmul(psv, lhsT=xnT[:, dc, ti * P:(ti + 1) * P],
                                     rhs=wv_sb[:, dc, fh * 512:(fh + 1) * 512],
                                     start=(dc == 0), stop=(dc == DC - 1))
                nc.vector.tensor_tensor(
                    v_aug[:, ti, fh * 8:(fh + 1) * 8, 0:DH],
                    psv[:].rearrange("p (h d) -> p h d", d=DH),
                    bv_b[:, fh * 512:(fh + 1) * 512].rearrange(
                        "p (h d) -> p h d", d=DH),
                    OP.add)

        for ti in range(TT):
            xn_t = lnxn.tile([P, D], bf16, tag="xn", name=uniq("xn"))
            layer_norm_tile(xfull[:, ti, :], xn_t)
            pe_transpose_to(xnT, ti, xn_t)
            if ti > 0:
                emit_v(ti - 1)
        emit_v(TT - 1)
        lnx.release()

        # ---- phase 3: attention, deep-pipelined across head pairs ----
        # Steady-state per-iteration engine streams (iteration i):
        #   PE : [b: av qh0(i) + sc qh1(i)] [QK(i+1)] [a: sc qh0(i+1)]
        #        [bcast qh0(i)] [c: av qh1(i)] [bcast qh1(i)]
        #   ACT: merged [128,1024] exps trail the score pairs; ACT stays
        #        saturated through phase c because iteration i+1's phase-a
        #        exps are already enqueued.
        #   DVE: den copies qh0 -> qk bias adds -> recip+mul qh0 ->
        #        den copies qh1 -> recip+mul qh1 (reciprocals run on the
        #        broadcast [64,512] tiles, entirely off the PE path).
        aoT = aop.tile([P, DC, N], bf16, tag="aoT")

        def new_qk_tiles():
            qt_t = qkp.tile([P, N], bf16, tag="qT", bufs=2, name=uniq("qT"))
            kt_t = qkp.tile([P, N], bf16, tag="kT", bufs=2, name=uniq("kT"))
            nc.vector.memset(qt_t[0:1, 0:4], 0.0)
            nc.vector.memset(kt_t[0:1, 0:4], 0.0)
            return qt_t, kt_t

        def emit_qk_half(hp, qh, qk):
            qt_t, kt_t = qk
            psqk = pse_tile()
            for dc in range(DC):
                nc.tensor.matmul(psqk[:, 0:512],
                                 lhsT=wq_sb[:, dc, hp * P:(hp + 1) * P],
                                 rhs=xnT[:, dc, qh * 512:(qh + 1) * 512],
                                 start=(dc == 0), stop=(dc == DC - 1))
            for dc in range(DC):
                nc.tensor.matmul(psqk[:, 512:1024],
                                 lhsT=wk_sb[:, dc, hp * P:(hp + 1) * P],
                                 rhs=xnT[:, dc, qh * 512:(qh + 1) * 512],
                                 start=(dc == 0), stop=(dc == DC - 1))
            nc.vector.tensor_scalar_add(qt_t[:, qh * 512:(qh + 1) * 512],
                                        psqk[:, 0:512], bq_sb[:, hp:hp + 1])
            nc.vector.tensor_scalar_add(kt_t[:, qh * 512:(qh + 1) * 512],
                                        psqk[:, 512:1024], bk_sb[:, hp:hp + 1])

        ext_of = {}

        def score_pair(qk, hp, mc, qh):
            qt_t, kt_t = qk
            pse = pse_tile()
            for hh, r0 in ((0, 0), (1, 64)):
                nc.tensor.matmul(
                    pse[:, hh * 512:(hh + 1) * 512],
                    lhsT=kt_t[r0:r0 + 64, mc * P:(mc + 1) * P],
                    rhs=qt_t[r0:r0 + 64, qh * 512:(qh + 1) * 512],
                    start=True, stop=True)
            ext = expp.tile([P, 1024], bf16, tag="expT", name=uniq("expT"))
            nc.scalar.activation(ext, pse, func=AF.Exp)
            ext_of[(hp, mc, qh)] = ext

        def emit_av(hp, mc, qh, hh, us):
            key = (hp, mc, qh)
            ext = ext_of[key] if hh == 0 else ext_of.pop(key)
            nc.tensor.matmul(
                us[qh][0:DH + 1, :],
                lhsT=v_aug[:, mc, 2 * hp + hh, :],
                rhs=ext[:, hh * 512:(hh + 1) * 512],
                start=(mc == 0), stop=(mc == TT - 1))

        den_init = [0]

        def den_reduce(usA, usB, qh):
            """Gather both heads' raw denominator rows (PSUM partition 64)
            onto quadrant-lane-0 rows of one SBUF tile, then take ONE wide
            reciprocal covering both (DVE time scales with free size only).
            Returns the fp32r reciprocal rows, ready for PE broadcast."""
            den_q = denp.tile([65, 512], f32, tag="denq", bufs=2,
                              name=uniq("denq"))
            den_r = denp.tile([65, 512], mybir.dt.float32r, tag="denr",
                              bufs=2, name=uniq("denr"))
            if den_init[0] < 2:
                # rows 1:63 are dead weight under the selector's zero rows,
                # but virgin-SBUF zeros would reciprocal to inf and 0*inf =
                # NaN in the broadcast matmul -- initialize both slots once
                # (later tenants inherit finite stale denominators)
                den_init[0] += 1
                nc.vector.memset(den_q[0:64, :], 1.0)
            with nc.allow_low_precision(
                    reason="fp32r rounding of the softmax denominator "
                           "reciprocal (~2^-13 relative) for the "
                           "1-cycle/row PE broadcast"):
                nc.vector.tensor_copy(den_q[0:1, :], usA[qh][DH:DH + 1, :])
                nc.vector.tensor_copy(den_q[64:65, :], usB[qh][DH:DH + 1, :])
                nc.vector.reciprocal(den_r, den_q)
            return den_r

        def stage_attnU(usA, usB, qh):
            """Drain unnormalized attn@V from PSUM to SBUF bf16 (a DVE
            tensor op may read at most ONE operand from PSUM, and the
            normalization multiply already reads the broadcast PSUM tile).
            Head hh=1 lands on partitions 64:128 so one [128,512] multiply
            covers the pair. Also frees the us accumulator banks early."""
            au = denp.tile([P, 512], bf16, tag="au", bufs=2, name=uniq("au"))
            with nc.allow_low_precision(
                    reason="bf16 staging of unnormalized attn output; "
                           "matches the bf16 precision of aoT itself"):
                nc.vector.tensor_copy(au[0:64, :], usA[qh][0:64, :])
                nc.vector.tensor_copy(au[64:128, :], usB[qh][0:64, :])
            return au

        def den_apply(au, hp, qh, den_r):
            """One K=65 selector matmul broadcasts both heads' reciprocal
            rows; one wide multiply normalizes the pair into aoT. The
            broadcast PSUM tile is drained by that fast multiply, so the
            pse slot recycles quickly (no reciprocal on the PE path)."""
            psb = ps_tile()
            nc.tensor.matmul(
                psb,
                lhsT=bones[:],
                rhs=den_r[:],
                start=True, stop=True)
            nc.vector.tensor_mul(
                aoT[:, hp, qh * 512:(qh + 1) * 512], au, psb)

        qk_cur = new_qk_tiles()
        emit_qk_half(0, 0, qk_cur)
        emit_qk_half(0, 1, qk_cur)
        for mc in range(TT):                # phase a(0)
            score_pair(qk_cur, 0, mc, 0)

        pending1 = None     # deferred qh=1 normalization of the previous hp
        for hp in range(DC):      # head pair: heads 2hp (rows 0:64), 2hp+1
            usA = [ps_tile() for _ in range(2)]
            usB = [ps_tile() for _ in range(2)]

            for mc in range(TT):            # phase b: attn@V qh=0 ; sc qh=1
                emit_av(hp, mc, 0, 0, usA)
                emit_av(hp, mc, 0, 1, usB)
                score_pair(qk_cur, hp, mc, 1)
            if pending1 is not None:
                den_apply(*pending1)        # bcast qh1(hp-1): recip long done
                pending1 = None
            den_r0 = den_reduce(usA, usB, 0)
            au0 = stage_attnU(usA, usB, 0)
            if hp + 1 < DC:
                qk_next = new_qk_tiles()
                emit_qk_half(hp + 1, 0, qk_next)
                emit_qk_half(hp + 1, 1, qk_next)
                # phase a(hp+1) scores interleaved with phase c avs: the avs
                # don't touch the pse ring, giving the ACT exps time to
                # drain each score pair's slot (no PE burst stall)
                for mc in range(TT):
                    score_pair(qk_next, hp + 1, mc, 0)
                    emit_av(hp, mc, 1, 0, usA)
                    emit_av(hp, mc, 1, 1, usB)
            else:
                for mc in range(TT):        # phase c: attn@V qh=1
                    emit_av(hp, mc, 1, 0, usA)
                    emit_av(hp, mc, 1, 1, usB)
            den_apply(au0, hp, 0, den_r0)
            den_r1 = den_reduce(usA, usB, 1)
            au1 = stage_attnU(usA, usB, 1)
            pending1 = (au1, hp, 1, den_r1)
            if hp + 1 < DC:
                qk_cur = qk_next

        expp.release()
        qkp.release()
        vap.release()
        xnTp.release()

        # q/k/v slots free -> start w1 loads (queue-aligned to old tenants)
        W1_PADS = globals().get("_W1_PADS", [0, 0, 0])
        w1_t = []
        for g in range(3):   # quarters 0-2 reuse the q/k/v slots right away
            qpad(W1_PADS[g])
            t = wtile([P, 2, MLP])
            nc.sync.dma_start(
                t, w1_d[g * 256:(g + 1) * 256, :].rearrange(
                    "(o p) f -> p o f", p=P))
            w1_t.append(t)

        # ---- phase 4: out-projection + LN2 + transpose ----
        hTp = tc.alloc_tile_pool(name="hT", bufs=1, side="right")
        xn2Tp = tc.alloc_tile_pool(name="xn2T", bufs=1, side="right")
        hT = hTp.tile([P, MT, N], bf16, tag="hT")
        xn2T = xn2Tp.tile([P, DC, N], bf16, tag="xn2T")

        pend_tr = [None]
        for ti in range(TT):
            x2_t = x2p.tile([P, D], bf16, tag="x2", name=uniq("x2"))
            nc.vector.memset(x2_t[0:1, 0:4], 0.0)
            for fh in range(2):
                pso = ps_tile()
                for dc in range(DC):
                    nc.tensor.matmul(pso, lhsT=aoT[:, dc, ti * P:(ti + 1) * P],
                                     rhs=wo_sb[:, dc, fh * 512:(fh + 1) * 512],
                                     start=(dc == 0), stop=(dc == DC - 1))
                nc.vector.tensor_add(x2_t[:, fh * 512:(fh + 1) * 512], pso,
                                     bo_b[:, fh * 512:(fh + 1) * 512])
            if ti == 0 and pending1 is not None:
                # last head pair's qh=1 normalization: its reciprocal ran
                # during ti=0's out-proj; ti=0 only reads qh=0 aoT columns
                den_apply(*pending1)
                pending1 = None
            # defer this tile's transposes behind the next tile's out-proj so
            # the in-order PE queue never waits on the DVE LayerNorm chain
            if pend_tr[0] is not None:
                pend_tr[0]()
            xn2_t = lnxn.tile([P, D], bf16, tag="xn", name=uniq("xn"))
            layer_norm_tile(x2_t, xn2_t)
            pend_tr[0] = (lambda ti=ti, xt=xn2_t: pe_transpose_to(
                xn2T, ti, xt))
        pend_tr[0]()

        # wout done -> load the last w1 quarter into its slot (4 chunks
        # along the mlp axis so FFN1's early m-tiles unblock quickly)
        qpad(globals().get("_W1D_PAD", 0))
        w1d = wtile([P, 2, MLP])
        for mq in range(4):
            nc.sync.dma_start(
                w1d[:, :, mq * 1024:(mq + 1) * 1024],
                w1_d[768:1024, mq * 1024:(mq + 1) * 1024].rearrange(
                    "(o p) f -> p o f", p=P))
        w1_t.append(w1d)

        x2p.release()
        denp.release()
        aop.release()
        lnxn.release()
        w2ep = tc.alloc_tile_pool(name="w2e", bufs=2)
        w2_t = []
        for g in range(2):
            t = w2ep.tile([P, TT, D], bf16, tag="w2e", name=uniq("w2e"))
            nc.sync.dma_start(
                t, w2_d[g * 1024:(g + 1) * 1024, :].rearrange(
                    "(o p) f -> p o f", p=P))
            w2_t.append(t)

        # ---- phase 5: FFN1 (feature-major h, fused bias+gelu) ----
        # both query-halves accumulate into one 2-bank tile; a single wide
        # ACT applies bias+gelu for the whole [128,1024] row block
        for m in range(MT):
            psh = pse_tile()
            for qh in range(2):
                for dc in range(DC):
                    nc.tensor.matmul(
                        psh[:, qh * 512:(qh + 1) * 512],
                        lhsT=w1_t[dc // 2][:, dc % 2, m * P:(m + 1) * P],
                        rhs=xn2T[:, dc, qh * 512:(qh + 1) * 512],
                        start=(dc == 0), stop=(dc == DC - 1))
            nc.scalar.activation(hT[:, m, :], psh,
                                 func=gelu_func, bias=bh_sb[:, m:m + 1],
                                 scale=1.0)

        # scrub w1 slots, load w2 quarters
        for g in range(2, 4):
            t = wtile([P, TT, D])
            nc.gpsimd.dma_start(
                t, w2_d[g * 1024:(g + 1) * 1024, :].rearrange(
                    "(o p) f -> p o f", p=P))
            w2_t.append(t)

        xn2Tp.release()

        # ---- phase 6: FFN2 (token-major out) ----
        # two t-tiles in flight (4 PSUM banks); bias-add + store of tile i
        # overlaps the 32-chunk accumulation of tile i+1, so only the last
        # tile's store is exposed at the end.
        # groups of 2 t-tiles, then singles at the end so the final exposed
        # add+store covers only one 512KB tile
        for tis in ((0, 1), (2, 3), (4, 5), (6,), (7,)):
            pss = [ps_tile() for _ in range(2 * len(tis))]
            for c in range(MT):
                for tloc, ti in enumerate(tis):
                    for fh in range(2):
                        nc.tensor.matmul(
                            pss[tloc * 2 + fh],
                            lhsT=hT[:, c, ti * P:(ti + 1) * P],
                            rhs=w2_t[c // 8][:, c % 8, fh * 512:(fh + 1) * 512],
                            start=(c == 0), stop=(c == MT - 1))
            for tloc, ti in enumerate(tis):
                o_t = outp.tile([P, D], f32, tag="o", name=uniq("o"))
                nc.vector.memset(o_t[0:1, 0:4], 0.0)
                for fh in range(2):
                    nc.vector.tensor_add(o_t[:, fh * 512:(fh + 1) * 512],
                                         pss[tloc * 2 + fh],
                                         b2_b[:, fh * 512:(fh + 1) * 512])
                nc.sync.dma_start(out_d[ti * P:(ti + 1) * P, :], o_t)

        w2ep.release()
        hTp.release()
        outp.release()
        wslot.release()
        psum.release()
        stats.release()
        const.release()

    nc.finalize()   # bacc legalization: wait splitting, table/library loads
    return nc


def prep_inputs(inputs):
    """Host-side weight folding + bf16 casts. Returns (shared_map, per_core_x)."""
    f = lambda k: np.asarray(inputs[k], dtype=np.float32)
    x = f("x")
    g1, b1 = f("ln1_g"), f("ln1_b")
    w_qkv, w_out, b_out = f("w_qkv"), f("w_out"), f("b_out")
    g2, b2l = f("ln2_g"), f("ln2_b")
    w1, bias1, w2, bias2 = f("w1"), f("b1"), f("w2"), f("b2")

    scale = DH ** -0.5
    wqkv_g = g1[:, None] * w_qkv
    bias_qkv = b1 @ w_qkv
    bf = ml_dtypes.bfloat16
    shared = {
        "wq": np.ascontiguousarray(wqkv_g[:, :D] * scale).astype(bf),
        "wk": np.ascontiguousarray(wqkv_g[:, D:2 * D]).astype(bf),
        "wv": np.ascontiguousarray(wqkv_g[:, 2 * D:]).astype(bf),
        "wo": w_out.astype(bf),
        "w1": (g2[:, None] * w1).astype(bf),
        "w2": w2.astype(bf),
        "bq": np.ascontiguousarray(bias_qkv[:D] * scale),
        "bk": np.ascontiguousarray(bias_qkv[D:2 * D]),
        "bv": np.ascontiguousarray(bias_qkv[2 * D:]),
        "bo": b_out.copy(),
        "bh": b2l @ w1 + bias1,
        "b2": bias2.copy(),
    }
    xs = [np.ascontiguousarray(x[i]).astype(bf) for i in range(B)]
    return shared, xs


_CACHED_NC = None


def _get_nc():
    global _CACHED_NC
    if _CACHED_NC is None:
        _CACHED_NC = build_bass()
    return _CACHED_NC


def run(inputs, trace=False):
    from concourse.bass_utils import run_bass_kernel_spmd
    nc = _get_nc()
    shared, xs = prep_inputs(inputs)
    in_maps = [{**shared, "x": xs[i]} for i in range(NCORES)]
    res = run_bass_kernel_spmd(nc, in_maps, list(range(NCORES)), trace=trace)
    out = np.stack([np.asarray(res.results[i]["out"]) for i in range(NCORES)], 0)
    return out.astype(np.float32), res


def kernel(**inputs):
    out, _ = run(inputs)
    return out



# revision 39
# speedup vs baseline: 1.2997x; 1.2997x over previous
"""Trainium2 Bass kernel for a dense transformer block (LN -> QKV -> attention ->
out-proj -> LN -> FFN with exact GELU, no residuals).

Sharding: pure data parallelism — batch 8 across 8 NeuronCores, one batch element
per core. Each core runs the full block on its [1024, 1024] token slab.

On-chip dataflow (per core):
  - LN1 in token-major fp32 (bn_stats); normalized output cast to bf16 and
    transposed to feature-major xnT [D, tokens] on the PE (bf16 transpose-mode).
  - All GEMMs in bf16 with fp32 PSUM accumulation. LayerNorm gains fold into the
    weights on the host; LN biases fold into per-feature GEMM biases. The
    1/sqrt(dh) attention scale folds into the Q projection.
  - Q/K produced feature-major per head pair (streamed), V token-major with a
    ones-column per head so attention@V also yields the softmax denominator.
  - Scores are computed k-major (scoresT) so the exp output feeds attention@V
    directly with no transpose; softmax skips max-subtraction (|scores| < ~3).
  - Denominator reciprocal is partition-broadcast by GPSIMD (no DMA).
  - x2 token-major, LN2 like LN1, FFN1 feature-major with fused bias+GELU on
    ACT, FFN2 token-major.

DMA discipline (hardware caps each DMA descriptor at ~2 sem waits, and Tile
adds one flow-control wait per DMA once queues saturate, so every DMA must
carry at most ONE data wait):
  - Weights live in 4 permanent 16KB "wslot" tag slots (wq/wk/wv/wout ->
    w1 quarters -> w2 quarters). Between tenants the slot is fully memset by
    GPSIMD: engine instructions have no wait caps and become the slot's sole
    last-writer, so the next load carries a single WAW wait.
  - Activation pools recycle SBUF freely but are only ever written by compute
    engines; output staging is a permanent pool so stores see only DVE deps.
"""

import numpy as np
import ml_dtypes

B, N, D = 8, 1024, 1024
H, DH = 16, 64
MLP = 4096
EPS = 1e-5
P = 128
NCORES = 8
TT = N // P    # 8 token tiles
DC = D // P    # 8 d-chunks
MT = MLP // P  # 32 mlp tiles


def build_bass(gelu_mode="gelu"):
    import concourse.bass as bass
    import concourse.mybir as mybir
    import concourse.tile as tile
    from concourse import bacc
    from concourse.masks import make_identity

    f32 = mybir.dt.float32
    bf16 = mybir.dt.bfloat16
    AF = mybir.ActivationFunctionType
    OP = mybir.AluOpType

    nc = bacc.Bacc()

    x_d = nc.declare_dram_parameter("x", [N, D], bf16, isOutput=False)
    wq_d = nc.declare_dram_parameter("wq", [D, D], bf16, isOutput=False)
    wk_d = nc.declare_dram_parameter("wk", [D, D], bf16, isOutput=False)
    wv_d = nc.declare_dram_parameter("wv", [D, D], bf16, isOutput=False)
    wo_d = nc.declare_dram_parameter("wo", [D, D], bf16, isOutput=False)
    w1_d = nc.declare_dram_parameter("w1", [D, MLP], bf16, isOutput=False)
    w2_d = nc.declare_dram_parameter("w2", [MLP, D], bf16, isOutput=False)
    bq_d = nc.declare_dram_parameter("bq", [D], f32, isOutput=False)
    bk_d = nc.declare_dram_parameter("bk", [D], f32, isOutput=False)
    bv_d = nc.declare_dram_parameter("bv", [D], f32, isOutput=False)
    bo_d = nc.declare_dram_parameter("bo", [D], f32, isOutput=False)
    bh_d = nc.declare_dram_parameter("bh", [MLP], f32, isOutput=False)
    b2_d = nc.declare_dram_parameter("b2", [D], f32, isOutput=False)
    out_d = nc.declare_dram_parameter("out", [N, D], f32, isOutput=True)

    gelu_func = AF.Gelu if gelu_mode == "gelu" else AF.Identity

    with tile.TileContext(nc) as tc:
        # ---- permanent pools (left stack bottom) ----
        const = tc.alloc_tile_pool(name="const", bufs=1)
        stats = tc.alloc_tile_pool(name="stats", bufs=4)
        psum = tc.alloc_tile_pool(name="psum", bufs=8, space="PSUM")
        wslot = tc.alloc_tile_pool(name="wslot", bufs=4)   # 4 x 16KB weight slots
        outp = tc.alloc_tile_pool(name="outp", bufs=2)

        counter = [0]

        def uniq(prefix):
            counter[0] += 1
            return f"{prefix}{counter[0]}"

        def ps_tile():
            return psum.tile([P, 512], f32, tag="ps", bufs=4, name=uniq("ps"))

        def pse_tile():
            # 2-bank [P,1024] tile: a row-tiled matmul pair writes its halves
            # (each within one bank) and a single wide ACT drains both.
            return psum.tile([P, 1024], f32, tag="pse", bufs=2,
                             name=uniq("pse"))

        def wtile(shape):
            return wslot.tile(shape, bf16, tag="w", name=uniq("w"))

        def qpad(n, after_ap=None):
            """Emit n dummy HWDGE DMAs (virgin tiny targets) to rotate the
            round-robin queue assignment, so the next real load lands on the
            same queue as its slot's previous tenant load (merging the
            slot-WAW wait with the prior-same-queue wait). after_ap pins the
            pads' schedule position via a RAW dependency."""
            for _ in range(n):
                if after_ap is None:
                    d = const.tile([1, 4], f32, tag=uniq("pad"),
                                   name=uniq("padt"))
                    nc.sync.dma_start(d, bq_d[None, 0:4])
                else:
                    d = const.tile([1, 4], bf16, tag=uniq("pad"),
                                   name=uniq("padt"))
                    nc.sync.dma_start(d, after_ap)

        eps_t = const.tile([P, 1], f32, tag="eps")
        nc.vector.memset(eps_t, EPS)
        bv_b = const.tile([P, D], bf16, tag="bvb")
        nc.gpsimd.dma_start(bv_b, bv_d[None, :].to_broadcast([P, D]))
        ident = const.tile([P, P], bf16, tag="ident")
        make_identity(nc, ident)
        # selector for the denominator broadcast: bones.T @ den_r replicates
        # den_r row 0 onto output partitions 0:64 and row 64 onto 64:128
        bonesf = const.tile([65, P], f32, tag="bonesf")
        nc.vector.memset(bonesf, 0.0)
        nc.vector.memset(bonesf[0:1, 0:64], 1.0)
        nc.vector.memset(bonesf[64:65, 64:128], 1.0)
        bones = const.tile([65, P], mybir.dt.float32r, tag="bones")
        nc.vector.tensor_copy(bones, bonesf)

        def layer_norm_tile(x_t, xn_t):
            """token-major [128, D] fp32 -> normalized bf16 (no gain/bias)."""
            nc.vector.memset(xn_t[0:1, 0:4], 0.0)   # claim slot: absorb WAR deps
            st = stats.tile([P, 2, 6], f32, tag="st", name=uniq("st"))
            xr = x_t.rearrange("p (s d) -> p s d", s=2)
            nc.vector.bn_stats(st[:, 0], xr[:, 0])
            nc.vector.bn_stats(st[:, 1], xr[:, 1])
            mv = stats.tile([P, 2], f32, tag="mv", name=uniq("mv"))
            nc.vector.bn_aggr(mv, st)
            rstd = stats.tile([P, 1], f32, tag="rstd", name=uniq("rstd"))
            nc.scalar.activation(rstd, mv[:, 1:2], func=AF.Sqrt, bias=eps_t,
                                 scale=1.0)
            nc.vector.reciprocal(rstd, rstd)
            nc.vector.tensor_scalar(xn_t, x_t, scalar1=mv[:, 0:1], scalar2=rstd,
                                    op0=OP.subtract, op1=OP.mult)

        def pe_transpose_to(dstT, ti, src_t):
            """[128 tok, D] bf16 -> feature-major dstT[:, dj, ti-slice].
            4 transposes share one PSUM tile so a single wide ACT copy
            drains them (one 352-cycle ACT overhead per 4, not per 1)."""
            for g in range(2):
                pst = psum.tile([P, 4, P], bf16, tag="ps", bufs=4,
                                name=uniq("pst"))
                for k in range(4):
                    dj = g * 4 + k
                    nc.tensor.transpose(pst[:, k, :],
                                        src_t[:, dj * P:(dj + 1) * P], ident)
                nc.scalar.activation(
                    dstT[:, g * 4:(g + 1) * 4, ti * P:(ti + 1) * P], pst,
                    func=AF.Copy)

        # ---- phase 1: LN1 + transpose to feature-major ----
        # left-stack pools in reverse-death order (LIFO); x slab on the right
        lnxn = tc.alloc_tile_pool(name="lnxn", bufs=2)     # dies after LN2
        aop = tc.alloc_tile_pool(name="ao", bufs=1)
        denp = tc.alloc_tile_pool(name="den", bufs=1)
        x2p = tc.alloc_tile_pool(name="x2", bufs=2)
        xnTp = tc.alloc_tile_pool(name="xnT", bufs=1)
        vap = tc.alloc_tile_pool(name="vaug", bufs=1)
        qkp = tc.alloc_tile_pool(name="qk", bufs=4)
        expp = tc.alloc_tile_pool(name="expp", bufs=18)
        lnx = tc.alloc_tile_pool(name="lnx", bufs=1, side="right")

        xnT = xnTp.tile([P, DC, N], bf16, tag="xnT")
        xfull = lnx.tile([P, TT, D], bf16, tag="x")
        # x tiles FIRST (LN1 blocks on tile 0), then weights in first-use
        # order (wv for the fused V projection, wq/wk at attention, wo last)
        # the sync HWDGE queue is SERIAL (~0.6us min per DMA) -- one DMA per
        # x tile, in strict first-need order: x0..x7, wv (V projection),
        # then wq/wk (attention) and wo; tiny bias loads last
        for xq in range(TT):
            nc.sync.dma_start(
                xfull[:, xq:xq + 1, :],
                x_d[xq * P:(xq + 1) * P, :].rearrange(
                    "(o p) f -> p o f", p=P))
        wv_sb = wtile([P, DC, D])
        nc.sync.dma_start(wv_sb, wv_d[:, :].rearrange("(o p) f -> p o f", p=P))
        wq_sb = wtile([P, DC, D])
        nc.sync.dma_start(wq_sb, wq_d[:, :].rearrange("(o p) f -> p o f", p=P))
        wk_sb = wtile([P, DC, D])
        nc.sync.dma_start(wk_sb, wk_d[:, :].rearrange("(o p) f -> p o f", p=P))
        wo_sb = wtile([P, DC, D])
        nc.sync.dma_start(wo_sb, wo_d[:, :].rearrange("(o p) f -> p o f", p=P))
        bq_sb = const.tile([P, DC], f32, tag="bq")
        nc.sync.dma_start(bq_sb, bq_d[:].rearrange("(o p) -> p o", p=P))
        bk_sb = const.tile([P, DC], f32, tag="bk")
        nc.sync.dma_start(bk_sb, bk_d[:].rearrange("(o p) -> p o", p=P))
        bh_sb = const.tile([P, MT], f32, tag="bh")
        nc.sync.dma_start(bh_sb, bh_d[:].rearrange("(o p) -> p o", p=P))
        bo_b = const.tile([P, D], bf16, tag="bob")
        nc.gpsimd.dma_start(bo_b, bo_d[None, :].to_broadcast([P, D]))
        b2_b = const.tile([P, D], bf16, tag="b2b")
        nc.gpsimd.dma_start(b2_b, b2_d[None, :].to_broadcast([P, D]))
        # V projection for tile ti needs only tile ti's transposes, so it is
        # fused one iteration behind LN1: the PE runs V matmuls while the DVE
        # does the next tile's LayerNorm (PE-dense from kernel start).
        v_aug = vap.tile([P, TT, H, DH + 1], bf16, tag="vaug")
        nc.vector.memset(v_aug[:, :, :, DH:DH + 1], 1.0)

        def emit_v(ti):
            for fh in range(2):
                psv = ps_tile()
                for dc in range(DC):
                    nc.tensor.matmul(psv, lhsT=xnT[:, dc, ti * P:(ti + 1) * P],
                                     rhs=wv_sb[:, dc, fh * 512:(fh + 1) * 512],
                                     start=(dc == 0), stop=(dc == DC - 1))
                nc.vector.tensor_tensor(
                    v_aug[:, ti, fh * 8:(fh + 1) * 8, 0:DH],
                    psv[:].rearrange("p (h d) -> p h d", d=DH),
                    bv_b[:, fh * 512:(fh + 1) * 512].rearrange(
                        "p (h d) -> p h d", d=DH),
                    OP.add)

        for ti in range(TT):
            xn_t = lnxn.tile([P, D], bf16, tag="xn", name=uniq("xn"))
            layer_norm_tile(xfull[:, ti, :], xn_t)
            pe_transpose_to(xnT, ti, xn_t)
            if ti > 0:
                emit_v(ti - 1)
        emit_v(TT - 1)
        lnx.release()

        # ---- phase 3: attention, deep-pipelined across head pairs ----
        # Steady-state per-iteration engine streams (iteration i):
        #   PE : [b: av qh0(i) + sc qh1(i)] [QK(i+1)] [a: sc qh0(i+1)]
        #        [bcast qh0(i)] [c: av qh1(i)] [bcast qh1(i)]
        #   ACT: merged [128,1024] exps trail the score pairs; ACT stays
        #        saturated through phase c because iteration i+1's phase-a
        #        exps are already enqueued.
        #   DVE: den copies qh0 -> qk bias adds -> recip+mul qh0 ->
        #        den copies qh1 -> recip+mul qh1 (reciprocals run on the
        #        broadcast [64,512] tiles, entirely off the PE path).
        aoT = aop.tile([P, DC, N], bf16, tag="aoT")

        def new_qk_tiles():
            qt_t = qkp.tile([P, N], bf16, tag="qT", bufs=2, name=uniq("qT"))
            kt_t = qkp.tile([P, N], bf16, tag="kT", bufs=2, name=uniq("kT"))
            nc.vector.memset(qt_t[0:1, 0:4], 0.0)
            nc.vector.memset(kt_t[0:1, 0:4], 0.0)
            return qt_t, kt_t

        def emit_qk_half(hp, qh, qk):
            qt_t, kt_t = qk
            psqk = pse_tile()
            for dc in range(DC):
                nc.tensor.matmul(psqk[:, 0:512],
                                 lhsT=wq_sb[:, dc, hp * P:(hp + 1) * P],
                                 rhs=xnT[:, dc, qh * 512:(qh + 1) * 512],
                                 start=(dc == 0), stop=(dc == DC - 1))
            for dc in range(DC):
                nc.tensor.matmul(psqk[:, 512:1024],
                                 lhsT=wk_sb[:, dc, hp * P:(hp + 1) * P],
                                 rhs=xnT[:, dc, qh * 512:(qh + 1) * 512],
                                 start=(dc == 0), stop=(dc == DC - 1))
            nc.vector.tensor_scalar_add(qt_t[:, qh * 512:(qh + 1) * 512],
                                        psqk[:, 0:512], bq_sb[:, hp:hp + 1])
            nc.vector.tensor_scalar_add(kt_t[:, qh * 512:(qh + 1) * 512],
                                        psqk[:, 512:1024], bk_sb[:, hp:hp + 1])

        ext_of = {}

        def score_pair(qk, hp, mc, qh):
            qt_t, kt_t = qk
            pse = pse_tile()
            for hh, r0 in ((0, 0), (1, 64)):
                nc.tensor.matmul(
                    pse[:, hh * 512:(hh + 1) * 512],
                    lhsT=kt_t[r0:r0 + 64, mc * P:(mc + 1) * P],
                    rhs=qt_t[r0:r0 + 64, qh * 512:(qh + 1) * 512],
                    start=True, stop=True)
            ext = expp.tile([P, 1024], bf16, tag="expT", name=uniq("expT"))
            nc.scalar.activation(ext, pse, func=AF.Exp)
            ext_of[(hp, mc, qh)] = ext

        def emit_av(hp, mc, qh, hh, us):
            key = (hp, mc, qh)
            ext = ext_of[key] if hh == 0 else ext_of.pop(key)
            nc.tensor.matmul(
                us[qh][0:DH + 1, :],
                lhsT=v_aug[:, mc, 2 * hp + hh, :],
                rhs=ext[:, hh * 512:(hh + 1) * 512],
                start=(mc == 0), stop=(mc == TT - 1))

        den_init = [0]

        def den_reduce(usA, usB, qh):
            """Gather both heads' raw denominator rows (PSUM partition 64)
            onto quadrant-lane-0 rows of one SBUF tile, then take ONE wide
            reciprocal covering both (DVE time scales with free size only).
            Returns the fp32r reciprocal rows, ready for PE broadcast."""
            den_q = denp.tile([65, 512], f32, tag="denq", bufs=2,
                              name=uniq("denq"))
            den_r = denp.tile([65, 512], mybir.dt.float32r, tag="denr",
                              bufs=2, name=uniq("denr"))
            if den_init[0] < 2:
                # rows 1:63 are dead weight under the selector's zero rows,
                # but virgin-SBUF zeros would reciprocal to inf and 0*inf =
                # NaN in the broadcast matmul -- initialize both slots once
                # (later tenants inherit finite stale denominators)
                den_init[0] += 1
                nc.vector.memset(den_q[0:64, :], 1.0)
            with nc.allow_low_precision(
                    reason="fp32r rounding of the softmax denominator "
                           "reciprocal (~2^-13 relative) for the "
                           "1-cycle/row PE broadcast"):
                nc.vector.tensor_copy(den_q[0:1, :], usA[qh][DH:DH + 1, :])
                nc.vector.tensor_copy(den_q[64:65, :], usB[qh][DH:DH + 1, :])
                nc.vector.reciprocal(den_r, den_q)
            return den_r

        def stage_attnU(usA, usB, qh):
            """Drain unnormalized attn@V from PSUM to SBUF bf16 (a DVE
            tensor op may read at most ONE operand from PSUM, and the
            normalization multiply already reads the broadcast PSUM tile).
            Head hh=1 lands on partitions 64:128 so one [128,512] multiply
            covers the pair. Also frees the us accumulator banks early."""
            au = denp.tile([P, 512], bf16, tag="au", bufs=2, name=uniq("au"))
            with nc.allow_low_precision(
                    reason="bf16 staging of unnormalized attn output; "
                           "matches the bf16 precision of aoT itself"):
                nc.vector.tensor_copy(au[0:64, :], usA[qh][0:64, :])
                nc.vector.tensor_copy(au[64:128, :], usB[qh][0:64, :])
            return au

        def den_apply(au, hp, qh, den_r):
            """One K=65 selector matmul broadcasts both heads' reciprocal
            rows; one wide multiply normalizes the pair into aoT. The
            broadcast PSUM tile is drained by that fast multiply, so the
            pse slot recycles quickly (no reciprocal on the PE path)."""
            psb = pse_tile()
            nc.tensor.matmul(
                psb[:, 0:512],
                lhsT=bones[:],
                rhs=den_r[:],
                start=True, stop=True)
            nc.vector.tensor_mul(
                aoT[:, hp, qh * 512:(qh + 1) * 512], au, psb[:, 0:512])

        qk_cur = new_qk_tiles()
        emit_qk_half(0, 0, qk_cur)
        emit_qk_half(0, 1, qk_cur)
        for mc in range(TT):                # phase a(0)
            score_pair(qk_cur, 0, mc, 0)

        pending1 = None     # deferred qh=1 normalization of the previous hp
        for hp in range(DC):      # head pair: heads 2hp (rows 0:64), 2hp+1
            usA = [ps_tile() for _ in range(2)]
            usB = [ps_tile() for _ in range(2)]

            for mc in range(TT):            # phase b: attn@V qh=0 ; sc qh=1
                emit_av(hp, mc, 0, 0, usA)
                emit_av(hp, mc, 0, 1, usB)
                score_pair(qk_cur, hp, mc, 1)
            if pending1 is not None:
                den_apply(*pending1)        # bcast qh1(hp-1): recip long done
                pending1 = None
            den_r0 = den_reduce(usA, usB, 0)
            au0 = stage_attnU(usA, usB, 0)
            if hp + 1 < DC:
                qk_next = new_qk_tiles()
                emit_qk_half(hp + 1, 0, qk_next)
                emit_qk_half(hp + 1, 1, qk_next)
                # phase a(hp+1) scores interleaved with phase c avs: the avs
                # don't touch the pse ring, giving the ACT exps time to
                # drain each score pair's slot (no PE burst stall)
                for mc in range(TT):
                    score_pair(qk_next, hp + 1, mc, 0)
                    emit_av(hp, mc, 1, 0, usA)
                    emit_av(hp, mc, 1, 1, usB)
            else:
                for mc in range(TT):        # phase c: attn@V qh=1
                    emit_av(hp, mc, 1, 0, usA)
                    emit_av(hp, mc, 1, 1, usB)
            den_apply(au0, hp, 0, den_r0)
            den_r1 = den_reduce(usA, usB, 1)
            au1 = stage_attnU(usA, usB, 1)
            pending1 = (au1, hp, 1, den_r1)
            if hp + 1 < DC:
                qk_cur = qk_next

        expp.release()
        qkp.release()
        vap.release()
        xnTp.release()

        # q/k/v slots free -> start w1 loads (queue-aligned to old tenants)
        W1_PADS = globals().get("_W1_PADS", [0, 0, 0])
        w1_t = []
        for g in range(3):   # quarters 0-2 reuse the q/k/v slots right away
            qpad(W1_PADS[g])
            t = wtile([P, 2, MLP])
            nc.sync.dma_start(
                t, w1_d[g * 256:(g + 1) * 256, :].rearrange(
                    "(o p) f -> p o f", p=P))
            w1_t.append(t)

        # ---- phase 4: out-projection + LN2 + transpose ----
        hTp = tc.alloc_tile_pool(name="hT", bufs=1, side="right")
        xn2Tp = tc.alloc_tile_pool(name="xn2T", bufs=1, side="right")
        hT = hTp.tile([P, MT, N], bf16, tag="hT")
        xn2T = xn2Tp.tile([P, DC, N], bf16, tag="xn2T")

        pend_tr = [None]
        for ti in range(TT):
            x2_t = x2p.tile([P, D], bf16, tag="x2", name=uniq("x2"))
            nc.vector.memset(x2_t[0:1, 0:4], 0.0)
            for fh in range(2):
                pso = ps_tile()
                for dc in range(DC):
                    nc.tensor.matmul(pso, lhsT=aoT[:, dc, ti * P:(ti + 1) * P],
                                     rhs=wo_sb[:, dc, fh * 512:(fh + 1) * 512],
                                     start=(dc == 0), stop=(dc == DC - 1))
                nc.vector.tensor_add(x2_t[:, fh * 512:(fh + 1) * 512], pso,
                                     bo_b[:, fh * 512:(fh + 1) * 512])
            if ti == 0 and pending1 is not None:
                # last head pair's qh=1 normalization: its reciprocal ran
                # during ti=0's out-proj; ti=0 only reads qh=0 aoT columns
                den_apply(*pending1)
                pending1 = None
            # defer this tile's transposes behind the next tile's out-proj so
            # the in-order PE queue never waits on the DVE LayerNorm chain
            if pend_tr[0] is not None:
                pend_tr[0]()
            xn2_t = lnxn.tile([P, D], bf16, tag="xn", name=uniq("xn"))
            layer_norm_tile(x2_t, xn2_t)
            pend_tr[0] = (lambda ti=ti, xt=xn2_t: pe_transpose_to(
                xn2T, ti, xt))
        pend_tr[0]()

        # wout done -> load the last w1 quarter into its slot (4 chunks
        # along the mlp axis so FFN1's early m-tiles unblock quickly)
        qpad(globals().get("_W1D_PAD", 0))
        w1d = wtile([P, 2, MLP])
        for mq in range(4):
            nc.sync.dma_start(
                w1d[:, :, mq * 1024:(mq + 1) * 1024],
                w1_d[768:1024, mq * 1024:(mq + 1) * 1024].rearrange(
                    "(o p) f -> p o f", p=P))
        w1_t.append(w1d)

        x2p.release()
        denp.release()
        aop.release()
        lnxn.release()
        w2ep = tc.alloc_tile_pool(name="w2e", bufs=2)
        w2_t = []
        for g in range(2):
            t = w2ep.tile([P, TT, D], bf16, tag="w2e", name=uniq("w2e"))
            nc.sync.dma_start(
                t, w2_d[g * 1024:(g + 1) * 1024, :].rearrange(
                    "(o p) f -> p o f", p=P))
            w2_t.append(t)

        # ---- phase 5: FFN1 (feature-major h, fused bias+gelu) ----
        # both query-halves accumulate into one 2-bank tile; a single wide
        # ACT applies bias+gelu for the whole [128,1024] row block
        for m in range(MT):
            psh = pse_tile()
            for qh in range(2):
                for dc in range(DC):
                    nc.tensor.matmul(
                        psh[:, qh * 512:(qh + 1) * 512],
                        lhsT=w1_t[dc // 2][:, dc % 2, m * P:(m + 1) * P],
                        rhs=xn2T[:, dc, qh * 512:(qh + 1) * 512],
                        start=(dc == 0), stop=(dc == DC - 1))
            nc.scalar.activation(hT[:, m, :], psh,
                                 func=gelu_func, bias=bh_sb[:, m:m + 1],
                                 scale=1.0)

        # scrub w1 slots, load w2 quarters
        for g in range(2, 4):
            t = wtile([P, TT, D])
            nc.gpsimd.dma_start(
                t, w2_d[g * 1024:(g + 1) * 1024, :].rearrange(
                    "(o p) f -> p o f", p=P))
            w2_t.append(t)

        xn2Tp.release()

        # ---- phase 6: FFN2 (token-major out) ----
        # two t-tiles in flight (4 PSUM banks); bias-add + store of tile i
        # overlaps the 32-chunk accumulation of tile i+1, so only the last
        # tile's store is exposed at the end.
        # groups of 2 t-tiles, then singles at the end so the final exposed
        # add+store covers only one 512KB tile
        for tis in ((0, 1), (2, 3), (4, 5), (6,), (7,)):
            pss = [ps_tile() for _ in range(2 * len(tis))]
            for c in range(MT):
                for tloc, ti in enumerate(tis):
                    for fh in range(2):
                        nc.tensor.matmul(
                            pss[tloc * 2 + fh],
                            lhsT=hT[:, c, ti * P:(ti + 1) * P],
                            rhs=w2_t[c // 8][:, c % 8, fh * 512:(fh + 1) * 512],
                            start=(c == 0), stop=(c == MT - 1))
            for tloc, ti in enumerate(tis):
                o_t = outp.tile([P, D], f32, tag="o", name=uniq("o"))
                nc.vector.memset(o_t[0:1, 0:4], 0.0)
                for fh in range(2):
                    nc.vector.tensor_add(o_t[:, fh * 512:(fh + 1) * 512],
                                         pss[tloc * 2 + fh],
                                         b2_b[:, fh * 512:(fh + 1) * 512])
                nc.sync.dma_start(out_d[ti * P:(ti + 1) * P, :], o_t)

        w2ep.release()
        hTp.release()
        outp.release()
        wslot.release()
        psum.release()
        stats.release()
        const.release()

    nc.finalize()   # bacc legalization: wait splitting, table/library loads
    return nc


def prep_inputs(inputs):
    """Host-side weight folding + bf16 casts. Returns (shared_map, per_core_x)."""
    f = lambda k: np.asarray(inputs[k], dtype=np.float32)
    x = f("x")
    g1, b1 = f("ln1_g"), f("ln1_b")
    w_qkv, w_out, b_out = f("w_qkv"), f("w_out"), f("b_out")
    g2, b2l = f("ln2_g"), f("ln2_b")
    w1, bias1, w2, bias2 = f("w1"), f("b1"), f("w2"), f("b2")

    scale = DH ** -0.5
    wqkv_g = g1[:, None] * w_qkv
    bias_qkv = b1 @ w_qkv
    bf = ml_dtypes.bfloat16
    shared = {
        "wq": np.ascontiguousarray(wqkv_g[:, :D] * scale).astype(bf),
        "wk": np.ascontiguousarray(wqkv_g[:, D:2 * D]).astype(bf),
        "wv": np.ascontiguousarray(wqkv_g[:, 2 * D:]).astype(bf),
        "wo": w_out.astype(bf),
        "w1": (g2[:, None] * w1).astype(bf),
        "w2": w2.astype(bf),
        "bq": np.ascontiguousarray(bias_qkv[:D] * scale),
        "bk": np.ascontiguousarray(bias_qkv[D:2 * D]),
        "bv": np.ascontiguousarray(bias_qkv[2 * D:]),
        "bo": b_out.copy(),
        "bh": b2l @ w1 + bias1,
        "b2": bias2.copy(),
    }
    xs = [np.ascontiguousarray(x[i]).astype(bf) for i in range(B)]
    return shared, xs


_CACHED_NC = None


def _get_nc():
    global _CACHED_NC
    if _CACHED_NC is None:
        _CACHED_NC = build_bass()
    return _CACHED_NC


def run(inputs, trace=False):
    from concourse.bass_utils import run_bass_kernel_spmd
    nc = _get_nc()
    shared, xs = prep_inputs(inputs)
    in_maps = [{**shared, "x": xs[i]} for i in range(NCORES)]
    res = run_bass_kernel_spmd(nc, in_maps, list(range(NCORES)), trace=trace)
    out = np.stack([np.asarray(res.results[i]["out"]) for i in range(NCORES)], 0)
    return out.astype(np.float32), res


def kernel(**inputs):
    out, _ = run(inputs)
    return out

